# revision 28
# baseline (speedup 1.0000x reference)
"""AttnBlock (GroupNorm + 4-head self-attention + out-proj) on 8 trn2 cores.

Sharding: core = (batch b in 0..1) x (query-quarter qc in 0..3).
Each core computes the full pipeline for its batch and its 1024-query slice.

v2 design (vs the 426us fp32-QK baseline):
  - ALL matmuls run fp16 (fp32 streams 4 cycles/col on the PE; fp16 streams 1).
    x ships from host as fp16; GN folds into fp16 qkv weights; kT/qT/v are
    fp16; scores accumulate fp32 in PSUM.
  - QK^T row-tiles head pairs at tile_position (32h, 0) so the two K=32
    matmuls of a pair ride the same PE pass (fp16 weights occupy one row
    group, unlike fp32 hi/lo which spilled into the partner's rows).
  - exp is the hard floor (16.8M elems at 1 elem/cycle/lane on ACT): the
    score tiles are split between the Scalar engine (exact exp) and the
    Vector engine (Schraudolph 2^t: int16(t*1024 + 15316) bit-cast to fp16,
    one tensor_scalar from PSUM; +-3% sawtooth cancels in softmax renorm to
    ~7e-3 final rel err at a 44% share).
  - AV accumulates all 32 j-tiles directly in PSUM (start at jt0, stop at
    jt31) - no block merges on DVE. The ones column appended to v yields the
    softmax denominator for free.
  - rsqrt for GN uses the int-shift seed + Newton on DVE, so the ACT sqrt
    table never loads and the exp table (pre-warmed at t=0) stays resident.
Output slices are disjoint, so the host just concatenates 8 results.
"""

import numpy as np
from contextlib import ExitStack

import concourse.bass as bass
import concourse.mybir as mybir
import concourse.tile as tile
from concourse import bacc
from concourse.bass_utils import run_bass_kernel_spmd

F32 = mybir.dt.float32
FP16 = mybir.dt.float16
I16 = mybir.dt.int16
I32 = mybir.dt.int32
AF = mybir.ActivationFunctionType
ALU = mybir.AluOpType
AX = mybir.AxisListType

HEADS, DH = 4, 32
C = 128           # channels == HEADS*DH
S = 4096          # spatial f*h*w
IC = 1024         # queries per core
NJT = S // 128    # 32 key tiles
SCALE = DH ** -0.5
EPS = 1e-5
NG = 32           # groupnorm groups
GSIZE = (C // NG) * S  # elements per group

LOG2E = 1.4426950408889634
SCH_A = SCALE * LOG2E * 1024.0   # Schraudolph multiplier (raw score -> i16)
SCH_B = 15360.0 - 44.0           # exponent bias - relative-error centering

import os
_EXP_MODE = os.environ.get("K_EXP", "mixed")      # mixed | act
_VAUG_ENG = os.environ.get("K_VAUG", "act")       # act | dve
_RSQRT = os.environ.get("K_RSQRT", "bit")         # bit | act
_AV_ACCUM = os.environ.get("K_AV", "full")        # full | block
_STATS = os.environ.get("K_STATS", "ttr")         # ttr | base
_EPI = os.environ.get("K_EPI", "v2")              # v2 | base
_WARM = os.environ.get("K_WARM", "on")            # on | off
_SC16 = os.environ.get("K_SC16", "off")           # on: fp16 score tiles
_FILL = int(os.environ.get("K_FILL", "0"))        # PE-warming filler MMs/pair
_PFILL = int(os.environ.get("K_PFILL", "0"))      # prologue filler MM count


def _exp_eng(p, half):
    """Engine for exp tile `half` of pair p: 'A' (scalar/ACT) or 'V' (DVE).
    Fixed per score buffer: sa (half 0) always ACT, sb (half 1) always DVE.
    The QK matmul writing sb is emitted first each pair: its WAR dependency
    (the slower DVE read of the previous pair) resolves last, so the sa
    matmul queues right behind it and rides the same PE pass."""
    if _EXP_MODE == "act":
        return 'A'
    return 'A' if half == 0 else 'V'


def _build():
    nc = bacc.Bacc("TRN2", target_bir_lowering=False)
    d_xb = nc.declare_dram_parameter("xb", [C, S], FP16, isOutput=False)
    d_xq = nc.declare_dram_parameter("xq", [C, IC], FP16, isOutput=False)
    d_wqkv = nc.declare_dram_parameter("wqkvT", [C, 3 * C], F32, isOutput=False)
    d_wout = nc.declare_dram_parameter("woutT", [C, C], F32, isOutput=False)
    d_gam = nc.declare_dram_parameter("gam", [C, 1], F32, isOutput=False)
    d_bet = nc.declare_dram_parameter("bet", [C, 1], F32, isOutput=False)
    d_bout = nc.declare_dram_parameter("bout", [C, 1], F32, isOutput=False)
    d_gmap = nc.declare_dram_parameter("gmap", [C, NG], F32, isOutput=False)
    d_gmapT = nc.declare_dram_parameter("gmapT", [NG, C], F32, isOutput=False)
    d_bmap = nc.declare_dram_parameter("bmap", [C, 2 * C], FP16, isOutput=False)
    d_y = nc.declare_dram_parameter("y", [C, IC], F32, isOutput=True)

    with tile.TileContext(nc) as tc, ExitStack() as ctx:
        nv, ns, nt = nc.vector, nc.scalar, nc.tensor
        P = ctx.enter_context(tc.tile_pool(name="persist", bufs=1))
        EA = ctx.enter_context(tc.tile_pool(name="ea", bufs=6))
        EI = ctx.enter_context(tc.tile_pool(name="ei", bufs=6))

        # ---------------- loads (chunked so stats overlap the DMA) ------
        xb = P.tile([C, S], FP16, tag="xb")
        for chk in range(8):
            sl = slice(chk * 512, (chk + 1) * 512)
            nc.sync.dma_start(xb[:, sl], d_xb[:, sl])
        xq = P.tile([C, IC], FP16, tag="xq")
        nc.sync.dma_start(xq[:], d_xq[:])
        wq = P.tile([C, 3 * C], F32, tag="wq")
        nc.sync.dma_start(wq[:], d_wqkv[:])
        wo = P.tile([C, C], F32, tag="wo")
        nc.sync.dma_start(wo[:], d_wout[:])
        gbb = P.tile([C, 3], F32, tag="gbb")
        nc.sync.dma_start(gbb[:, 0:1], d_gam[:])
        nc.sync.dma_start(gbb[:, 1:2], d_bet[:])
        nc.sync.dma_start(gbb[:, 2:3], d_bout[:])
        gam, bet, bout = gbb[:, 0:1], gbb[:, 1:2], gbb[:, 2:3]
        gmap = P.tile([C, NG], F32, tag="gmap")
        nc.sync.dma_start(gmap[:], d_gmap[:])
        gmapT = P.tile([NG, C], F32, tag="gmapT")
        nc.sync.dma_start(gmapT[:], d_gmapT[:])
        bmap = P.tile([C, 2 * C], FP16, tag="bmap")
        nc.sync.dma_start(bmap[:], d_bmap[:])

        # persistent products
        kT = P.tile([C, S], FP16, tag="kT")        # [(h,d), j]
        qT = P.tile([C, IC], FP16, tag="qT")       # [(h,d), i]
        vaug = P.tile([C, NJT * HEADS * (DH + 1)], FP16, tag="vaug")
        vaug3 = vaug[:].rearrange("p (a b) -> p a b", b=DH + 1)  # a = jt*4+h
        wqs = P.tile([C, 3 * C], FP16, tag="wqs")
        wo16 = P.tile([C, C], FP16, tag="wo16")
        stat = P.tile([C, 4], F32, tag="stat")
        stat16 = P.tile([C, 16], F32, tag="stat16")
        gstat = P.tile([NG, 12], F32, tag="gstat")
        gsti = P.tile([NG, 2], I32, tag="gsti")
        qb = P.tile([C, 1], F32, tag="qb")
        kb = P.tile([C, 1], F32, tag="kb")
        vb = P.tile([C, 1], F32, tag="vb")
        ybias = P.tile([C, 1], F32, tag="ybias")
        warm = P.tile([C, 1], F32, tag="warm")
        d4 = P.tile([C, IC], F32, tag="d4")
        r4 = P.tile([C, IC], FP16, tag="r4")
        re01_sb = P.tile([C, IC], F32, tag="re01_sb")
        re23_sb = P.tile([C, IC], F32, tag="re23_sb")
        osc = P.tile([C, IC], FP16, tag="osc")
        ysb = P.tile([C, IC], F32, tag="ysb")

        if _PFILL or _FILL:
            zfull = P.tile([C, C], FP16, tag="zfull")
            nv.memset(zfull[:], 0.0)

        # pre-warm the exp activation table during the DMAs
        if _WARM == "on":
            nv.memset(warm[:], 0.0)
            ns.activation(warm[:], warm[:], AF.Exp, scale=1.0)

        # ---------------- prologue: GN stats + weight folding + qkv ----
        with tc.tile_pool(name="pps", bufs=2, space="PSUM") as PPS, \
             tc.tile_pool(name="wsc", bufs=2) as WSC:
            if _PFILL:
                # full-array zero passes during the DMA wait: PE cell
                # activity from t~0 so the HAM clock grant arrives early
                pfscr = PPS.tile([C, 512], F32, tag="pfscr")
                for i in range(_PFILL):
                    nt.matmul(pfscr[:, 0:128], zfull[:], zfull[:, 0:C],
                              start=True, stop=True, skip_group_check=True)
            # per-channel sum / sum-of-squares, chunked to overlap the DMA
            for chk in range(8):
                sl = slice(chk * 512, (chk + 1) * 512)
                nv.tensor_reduce(stat16[:, chk:chk + 1], xb[:, sl], AX.X,
                                 ALU.add)
                if _STATS == "ttr":
                    xsq = WSC.tile([C, 512], FP16, tag="xsq")
                    nv.tensor_tensor_reduce(
                        xsq[:], xb[:, sl], xb[:, sl], 1.0, 0.0,
                        ALU.mult, ALU.add, stat16[:, 8 + chk:9 + chk])
                else:
                    xsq = WSC.tile([C, 512], F32, tag="xsqf")
                    nv.tensor_mul(xsq[:], xb[:, sl], xb[:, sl])
                    nv.tensor_reduce(stat16[:, 8 + chk:9 + chk], xsq[:],
                                     AX.X, ALU.add)
            nv.tensor_reduce(stat[:, 0:1], stat16[:, 0:8], AX.X, ALU.add)
            nv.tensor_reduce(stat[:, 1:2], stat16[:, 8:16], AX.X, ALU.add)
            # group-combine via matmul with the group indicator matrix
            gs_p = PPS.tile([NG, 2], F32, tag="tiny")
            nt.matmul(gs_p[:], gmap[:], stat[:, 0:2], start=True, stop=True)
            m_g = gstat[:, 0:1]
            ex2 = gstat[:, 1:2]
            nv.tensor_scalar_mul(m_g, gs_p[:, 0:1], 1.0 / GSIZE)
            nv.tensor_scalar_mul(ex2, gs_p[:, 1:2], 1.0 / GSIZE)
            msq = gstat[:, 2:3]
            nv.tensor_mul(msq, m_g, m_g)
            vare = gstat[:, 3:4]
            nv.tensor_sub(vare, ex2, msq)
            nv.tensor_scalar_add(vare, vare, EPS)   # var + eps
            if _RSQRT == "bit":
                # rsqrt seed via the int-shift trick (no ACT sqrt table)
                nv.tensor_scalar(gsti[:, 0:1], vare.bitcast(I32), 1, None,
                                 ALU.logical_shift_right)
                nv.tensor_scalar(gsti[:, 1:2], gsti[:, 0:1], -1, 0x5f3759df,
                                 ALU.mult, ALU.add)
                r = gsti[:, 1:2].bitcast(F32)
            else:
                sq = gstat[:, 4:5]
                ns.activation(sq, vare, AF.Sqrt)
                r0 = gstat[:, 5:6]
                nv.reciprocal(r0, sq)
                r = r0
            # three Newton steps: r <- r * (1.5 - 0.5 * vare * r^2)
            for it in range(3):
                t1 = gstat[:, 6:7]
                nv.tensor_mul(t1, r, r)
                t2 = gstat[:, 7:8]
                nv.tensor_mul(t2, t1, vare)
                t3 = gstat[:, 8:9]
                nv.tensor_scalar(t3, t2, -0.5, 1.5, ALU.mult, ALU.add)
                rn = gstat[:, 9 + it:10 + it]
                nv.tensor_mul(rn, r, t3)
                r = rn
            # broadcast group mean/rstd back to channels
            st2 = gstat[:, 6:8]
            nv.tensor_copy(st2[:, 0:1], m_g)
            nv.tensor_copy(st2[:, 1:2], r)
            ch_p = PPS.tile([C, 2], F32, tag="tiny")
            nt.matmul(ch_p[:], gmapT[:], st2, start=True, stop=True)
            scale_c = stat[:, 2:3]
            nv.tensor_mul(scale_c, ch_p[:, 1:2], gam)
            tb = stat[:, 3:4]
            nv.tensor_mul(tb, ch_p[:, 0:1], scale_c)
            nv.tensor_sub(tb, bet, tb)

            # fold GN scale into qkv weights (fp16); out-proj weights fp16
            nv.tensor_scalar_mul(wqs[:], wq[:], scale_c)
            nv.tensor_copy(wo16[:], wo[:])
            # qkv biases from the GN shift
            for bi, btile in enumerate((qb, kb, vb)):
                bp = PPS.tile([C, 1], F32, tag="tiny")
                nt.matmul(bp[:], wq[:, bi * C:(bi + 1) * C], tb, start=True,
                          stop=True)
                nv.tensor_copy(btile[:], bp[:])
            ybp = PPS.tile([C, 1], F32, tag="tiny")
            nt.matmul(ybp[:], wo[:], vb[:], start=True, stop=True)
            nv.tensor_add(ybias[:], ybp[:], bout)

            # qT / kT (with folded bias), stacked [(h,d), *], fp16
            for chk in range(IC // 512):
                pq = PPS.tile([C, 512], F32, tag="pq")
                nt.matmul(pq[:], wqs[:, 0:C], xq[:, chk * 512:(chk + 1) * 512],
                          start=True, stop=True)
                nv.tensor_scalar_add(qT[:, chk * 512:(chk + 1) * 512], pq[:],
                                     qb[:])
            for chk in range(S // 512):
                pk = PPS.tile([C, 512], F32, tag="pq")
                nt.matmul(pk[:], wqs[:, C:2 * C], xb[:, chk * 512:(chk + 1) * 512],
                          start=True, stop=True)
                nv.tensor_scalar_add(kT[:, chk * 512:(chk + 1) * 512], pk[:],
                                     kb[:])
            # v in [j, (h,d)] fp16 layout with ones columns interleaved
            # (softmax denominator); 4 j-tiles batched per PSUM tile.
            # PSUM->SBUF copies go on ACT to keep DVE free for the kq casts.
            nv.memset(vaug3[:, :, DH:DH + 1], 1.0)
            for g in range(NJT // 4):
                pv = PPS.tile([C, 512], F32, tag="pq")
                for k in range(4):
                    nt.matmul(pv[:, k * 128:(k + 1) * 128],
                              xb[:, (4 * g + k) * 128:(4 * g + k + 1) * 128],
                              wqs[:, 2 * C:3 * C], start=True, stop=True)
                if _VAUG_ENG == "act":
                    ns.copy(vaug3[:, g * 16:(g + 1) * 16, 0:DH],
                            pv[:].rearrange("p (a d) -> p a d", d=DH))
                else:
                    nv.tensor_copy(vaug3[:, g * 16:(g + 1) * 16, 0:DH],
                                   pv[:].rearrange("p (a d) -> p a d", d=DH))

        # ---------------- attention ----------------
        # Pair structure: per (jt, pair): two score tiles sa (heads 2p) and
        # sb (2p+1), [128, 1024] each, ring of 2 (4 banks). sa always exps
        # on ACT, sb on DVE. The QK matmul writing sb is emitted first: its
        # write-after-read dependency (the slower DVE read of the previous
        # pair) resolves last, so the sa matmul queues behind it and rides
        # the same PE pass. AV lags two pairs so the exps' coalesced
        # PE-semaphore targets exclude AV matmuls. AV accumulates all 32
        # j-tiles in PSUM (no merge adds); the ones column in v gives the
        # softmax denominator for free.
        with tc.tile_pool(name="psc", bufs=2, space="PSUM") as PSC, \
             tc.tile_pool(name="pob", bufs=1, space="PSUM") as POB:
            ob01 = POB.tile([C, IC], F32, tag="ob01")
            ob23 = POB.tile([C, IC], F32, tag="ob23")

            def emit_fill(n):
                # full-array zero-accumulate passes (numeric no-op): keeps
                # the PE cell-activity high through exp handoffs so the HAM
                # clock gate grants full rate
                for i in range(n):
                    nt.matmul(ob01[:, 0:512], zfull[:], qT[0:C, 0:512],
                              start=False, stop=False, skip_group_check=True)

            def emit_exp(src_ap, eng, name):
                if _EXP_MODE == "act":
                    eng = 'A'
                if eng == 'A':
                    e = EA.tile([C, IC], FP16, tag="Ea", name=f"ea{name}")
                    ns.activation(e[:], src_ap, AF.Exp, scale=SCALE)
                    return ('A', e)
                e = EI.tile([C, IC], I16, tag="Ei", name=f"ei{name}")
                nv.tensor_scalar(e[:], src_ap, SCH_A, SCH_B,
                                 ALU.mult, ALU.add)
                return ('V', e)

            def av_rhs(ent, hf):
                kind, e = ent
                ap = e[:, hf * 512:(hf + 1) * 512]
                return ap.bitcast(FP16) if kind == 'V' else ap

            def emit_av(jt, pr, ea, eb):
                o = ob01 if pr == 0 else ob23
                first, last = jt == 0, jt == NJT - 1
                ha, hb = 2 * pr, 2 * pr + 1
                for hf in range(2):
                    for base, ent in ((0, ea), (64, eb)):
                        nt.matmul(o[base:base + DH + 1,
                                    hf * 512:(hf + 1) * 512],
                                  vaug3[:, jt * HEADS + (ha if base == 0
                                                         else hb), :],
                                  av_rhs(ent, hf),
                                  start=first, stop=last,
                                  skip_group_check=True,
                                  tile_position=(0, base))

            avq = []
            for jt in range(NJT):
                for pr in range(2):
                    p = 2 * jt + pr
                    ha, hb = 2 * pr, 2 * pr + 1
                    sa = PSC.tile([C, IC], F32, tag="sc", name=f"sa{p}")
                    sb = PSC.tile([C, IC], F32, tag="sc", name=f"sb{p}")
                    for hf in range(2):
                        for h, sc_ in ((hb, sb), (ha, sa)):
                            nt.matmul(sc_[:, hf * 512:(hf + 1) * 512],
                                      kT[32 * h:32 * (h + 1),
                                         jt * 128:(jt + 1) * 128],
                                      qT[32 * h:32 * (h + 1),
                                         hf * 512:(hf + 1) * 512],
                                      start=True, stop=True,
                                      tile_position=(32 * h, 0))
                    ea = emit_exp(sa[:], 'A', f"a{p}")
                    eb = emit_exp(sb[:], 'V', f"b{p}")
                    avq.append((jt, pr, ea, eb))
                    if _FILL and 1 < p < 63:
                        emit_fill(_FILL)
                    if len(avq) > 2:
                        emit_av(*avq.pop(0))
            while avq:
                emit_av(*avq.pop(0))

            # ---------------- epilogue ----------------
            if _AV_ACCUM == "block":
                ob01, ob23 = o01sb, o23sb
            # denominators land at ob partitions DH / 64+DH; gather them to
            # partitions 0/32/64/96 of d4 (rest stays 1.0 for the reciprocal)
            nv.memset(d4[:], 1.0)
            if _EPI == "v2":
                ns.copy(d4[0:1, :], ob01[DH:DH + 1, :])
                nv.tensor_copy(d4[32:33, :], ob01[64 + DH:64 + DH + 1, :])
                ns.copy(d4[64:65, :], ob23[DH:DH + 1, :])
                nv.tensor_copy(d4[96:97, :], ob23[64 + DH:64 + DH + 1, :])
                with nc.allow_low_precision("softmax recip in fp16 is fine"):
                    nv.reciprocal(r4[:], d4[:])
            else:
                nv.tensor_copy(d4[0:1, :], ob01[DH:DH + 1, :])
                nv.tensor_copy(d4[32:33, :], ob01[64 + DH:64 + DH + 1, :])
                nv.tensor_copy(d4[64:65, :], ob23[DH:DH + 1, :])
                nv.tensor_copy(d4[96:97, :], ob23[64 + DH:64 + DH + 1, :])
                r4f = P.tile([C, IC], F32, tag="r4f")
                nv.reciprocal(r4f[:], d4[:])
                nv.tensor_copy(r4[:], r4f[:])
            # broadcast reciprocals into the accumulators' own partition
            # layout (heads at rows 0:32 / 64:96 of each half) so the DVE
            # multiplies see equal base partitions on both inputs
            re01t = PSC.tile([C, IC], F32, tag="sc")
            re23t = PSC.tile([C, IC], F32, tag="sc")
            re01 = re01t[:]
            re23 = re23t[:]
            for hf in range(2):
                nt.matmul(re01t[:, hf * 512:(hf + 1) * 512], bmap[:, 0:C],
                          r4[:, hf * 512:(hf + 1) * 512], start=True, stop=True)
                nt.matmul(re23t[:, hf * 512:(hf + 1) * 512], bmap[:, C:2 * C],
                          r4[:, hf * 512:(hf + 1) * 512], start=True, stop=True)
            if _EPI == "v2":
                ns.copy(re01_sb[:], re01)
            else:
                nv.tensor_copy(re01_sb[:], re01)
            nv.tensor_copy(re23_sb[:], re23)
            nv.tensor_mul(osc[0:32, :], ob01[0:32, :], re01_sb[0:32, :])
            nv.tensor_mul(osc[32:64, :], ob01[64:96, :], re01_sb[64:96, :])
            nv.tensor_mul(osc[64:96, :], ob23[0:32, :], re23_sb[0:32, :])
            nv.tensor_mul(osc[96:128, :], ob23[64:96, :], re23_sb[64:96, :])
            ypsum = POB.tile([C, IC], F32,
                             tag=("ob01" if _AV_ACCUM == "full" else "ob"))
            for hf in range(2):
                nt.matmul(ypsum[:, hf * 512:(hf + 1) * 512], wo16[:],
                          osc[:, hf * 512:(hf + 1) * 512], start=True,
                          stop=True)
            nv.tensor_scalar_add(ysb[:], ypsum[:], ybias[:])
            nc.sync.dma_start(d_y[:], ysb[:])

    nc.compile()   # bacc passes: split sync waits (HW: 1 wait/inst), DCE
    return nc


_PROG = None


def _get_prog():
    global _PROG
    if _PROG is None:
        _PROG = _build()
    return _PROG


def _in_maps(x, gn_gamma, gn_beta, w_qkv, w_out, b_out):
    x = np.asarray(x, dtype=np.float32)
    gmap = np.zeros((C, NG), dtype=np.float32)
    gmap[np.arange(C), np.arange(C) // (C // NG)] = 1.0
    # cols 0:128 build re01 (h0 -> rows 0:32, h1 -> rows 64:96),
    # cols 128:256 build re23 (h2 -> rows 0:32, h3 -> rows 64:96);
    # reciprocal sources sit at partitions 0/32/64/96 of r4
    bmap = np.zeros((C, 2 * C), dtype=np.float16)
    bmap[0, 0:32] = 1.0
    bmap[32, 64:96] = 1.0
    bmap[64, C + 0:C + 32] = 1.0
    bmap[96, C + 64:C + 96] = 1.0
    base = dict(
        wqkvT=np.ascontiguousarray(np.asarray(w_qkv, np.float32).T),
        woutT=np.ascontiguousarray(np.asarray(w_out, np.float32).T),
        gam=np.asarray(gn_gamma, np.float32).reshape(C, 1),
        bet=np.asarray(gn_beta, np.float32).reshape(C, 1),
        bout=np.asarray(b_out, np.float32).reshape(C, 1),
        gmap=gmap,
        gmapT=np.ascontiguousarray(gmap.T),
        bmap=bmap,
    )
    maps = []
    for core in range(8):
        b, qc = core // 4, core % 4
        xb = np.ascontiguousarray(x[b].reshape(C, S).astype(np.float16))
        m = dict(base)
        m["xb"] = xb
        m["xq"] = np.ascontiguousarray(xb[:, qc * IC:(qc + 1) * IC])
        maps.append(m)
    return maps


def kernel(x, gn_gamma, gn_beta, w_qkv, w_out, b_out):
    nc = _get_prog()
    maps = _in_maps(x, gn_gamma, gn_beta, w_qkv, w_out, b_out)
    res = run_bass_kernel_spmd(nc, maps, list(range(8))).results
    y = np.empty((2, C, S), dtype=np.float32)
    for core in range(8):
        b, qc = core // 4, core % 4
        y[b, :, qc * IC:(qc + 1) * IC] = res[core]["y"]
    return y.reshape(2, C, 16, 16, 16)


# revision 30
# speedup vs baseline: 1.2055x; 1.2055x over previous
"""AttnBlock (GroupNorm + 4-head self-attention + out-proj) on 8 trn2 cores.

Sharding: core = (batch b in 0..1) x (query-quarter qc in 0..3).
Each core computes the full pipeline for its batch and its 1024-query slice.

v2 design (vs the 426us fp32-QK baseline):
  - ALL matmuls run fp16 (fp32 streams 4 cycles/col on the PE; fp16 streams 1).
    x ships from host as fp16; GN folds into fp16 qkv weights; kT/qT/v are
    fp16; scores accumulate fp32 in PSUM.
  - QK^T row-tiles head pairs at tile_position (32h, 0) so the two K=32
    matmuls of a pair ride the same PE pass (fp16 weights occupy one row
    group, unlike fp32 hi/lo which spilled into the partner's rows).
  - exp is the hard floor (16.8M elems at 1 elem/cycle/lane on ACT): the
    score tiles are split between the Scalar engine (exact exp) and the
    Vector engine (Schraudolph 2^t: int16(t*1024 + 15316) bit-cast to fp16,
    one tensor_scalar from PSUM; +-3% sawtooth cancels in softmax renorm to
    ~7e-3 final rel err at a 44% share).
  - AV accumulates all 32 j-tiles directly in PSUM (start at jt0, stop at
    jt31) - no block merges on DVE. The ones column appended to v yields the
    softmax denominator for free.
  - rsqrt for GN uses the int-shift seed + Newton on DVE, so the ACT sqrt
    table never loads and the exp table (pre-warmed at t=0) stays resident.
Output slices are disjoint, so the host just concatenates 8 results.
"""

import numpy as np
from contextlib import ExitStack

import concourse.bass as bass
import concourse.mybir as mybir
import concourse.tile as tile
from concourse import bacc
from concourse.bass_utils import run_bass_kernel_spmd

F32 = mybir.dt.float32
FP16 = mybir.dt.float16
I16 = mybir.dt.int16
I32 = mybir.dt.int32
AF = mybir.ActivationFunctionType
ALU = mybir.AluOpType
AX = mybir.AxisListType

HEADS, DH = 4, 32
C = 128           # channels == HEADS*DH
S = 4096          # spatial f*h*w
IC = 1024         # queries per core
NJT = S // 128    # 32 key tiles
SCALE = DH ** -0.5
EPS = 1e-5
NG = 32           # groupnorm groups
GSIZE = (C // NG) * S  # elements per group

LOG2E = 1.4426950408889634
SCH_A = SCALE * LOG2E * 1024.0   # Schraudolph multiplier (raw score -> i16)
SCH_B = 15360.0 - 44.0           # exponent bias - relative-error centering

import os
_EXP_MODE = os.environ.get("K_EXP", "mixed")      # mixed | act
_VAUG_ENG = os.environ.get("K_VAUG", "act")       # act | dve
_RSQRT = os.environ.get("K_RSQRT", "bit")         # bit | act
_AV_ACCUM = os.environ.get("K_AV", "full")        # full | block
_STATS = os.environ.get("K_STATS", "ttr")         # ttr | base
_EPI = os.environ.get("K_EPI", "v2")              # v2 | base
_WARM = os.environ.get("K_WARM", "on")            # on | off
_SC16 = os.environ.get("K_SC16", "off")           # on: fp16 score tiles
_FILL = int(os.environ.get("K_FILL", "0"))        # PE-warming filler MMs/pair
_PFILL = int(os.environ.get("K_PFILL", "0"))      # prologue filler MM count


def _exp_eng(p, half):
    """Engine for exp tile `half` of pair p: 'A' (scalar/ACT) or 'V' (DVE).
    Fixed per score buffer: sa (half 0) always ACT, sb (half 1) always DVE.
    The QK matmul writing sb is emitted first each pair: its WAR dependency
    (the slower DVE read of the previous pair) resolves last, so the sa
    matmul queues right behind it and rides the same PE pass."""
    if _EXP_MODE == "act":
        return 'A'
    return 'A' if half == 0 else 'V'


def _build():
    nc = bacc.Bacc("TRN2", target_bir_lowering=False)
    d_xb = nc.declare_dram_parameter("xb", [C, S], FP16, isOutput=False)
    d_xq = nc.declare_dram_parameter("xq", [C, IC], FP16, isOutput=False)
    d_wqkv = nc.declare_dram_parameter("wqkvT", [C, 3 * C], F32, isOutput=False)
    d_wout = nc.declare_dram_parameter("woutT", [C, C], F32, isOutput=False)
    d_gam = nc.declare_dram_parameter("gam", [C, 1], F32, isOutput=False)
    d_bet = nc.declare_dram_parameter("bet", [C, 1], F32, isOutput=False)
    d_bout = nc.declare_dram_parameter("bout", [C, 1], F32, isOutput=False)
    d_gmap = nc.declare_dram_parameter("gmap", [C, NG], F32, isOutput=False)
    d_gmapT = nc.declare_dram_parameter("gmapT", [NG, C], F32, isOutput=False)
    d_bmap = nc.declare_dram_parameter("bmap", [C, 2 * C], FP16, isOutput=False)
    d_y = nc.declare_dram_parameter("y", [C, IC], F32, isOutput=True)

    with tile.TileContext(nc) as tc, ExitStack() as ctx:
        nv, ns, nt = nc.vector, nc.scalar, nc.tensor
        P = ctx.enter_context(tc.tile_pool(name="persist", bufs=1))
        EA = ctx.enter_context(tc.tile_pool(name="ea", bufs=6))
        EI = ctx.enter_context(tc.tile_pool(name="ei", bufs=6))

        # ---------------- loads (chunked so stats overlap the DMA) ------
        xb = P.tile([C, S], FP16, tag="xb")
        for chk in range(8):
            sl = slice(chk * 512, (chk + 1) * 512)
            nc.sync.dma_start(xb[:, sl], d_xb[:, sl])
        xq = P.tile([C, IC], FP16, tag="xq")
        nc.sync.dma_start(xq[:], d_xq[:])
        wq = P.tile([C, 3 * C], F32, tag="wq")
        nc.sync.dma_start(wq[:], d_wqkv[:])
        wo = P.tile([C, C], F32, tag="wo")
        nc.sync.dma_start(wo[:], d_wout[:])
        gbb = P.tile([C, 3], F32, tag="gbb")
        nc.sync.dma_start(gbb[:, 0:1], d_gam[:])
        nc.sync.dma_start(gbb[:, 1:2], d_bet[:])
        nc.sync.dma_start(gbb[:, 2:3], d_bout[:])
        gam, bet, bout = gbb[:, 0:1], gbb[:, 1:2], gbb[:, 2:3]
        gmap = P.tile([C, NG], F32, tag="gmap")
        nc.sync.dma_start(gmap[:], d_gmap[:])
        gmapT = P.tile([NG, C], F32, tag="gmapT")
        nc.sync.dma_start(gmapT[:], d_gmapT[:])
        bmap = P.tile([C, 2 * C], FP16, tag="bmap")
        nc.sync.dma_start(bmap[:], d_bmap[:])

        # persistent products
        kT = P.tile([C, S], FP16, tag="kT")        # [(h,d), j]
        qT = P.tile([C, IC], FP16, tag="qT")       # [(h,d), i]
        vaug = P.tile([C, NJT * HEADS * (DH + 1)], FP16, tag="vaug")
        vaug3 = vaug[:].rearrange("p (a b) -> p a b", b=DH + 1)  # a = jt*4+h
        wqs = P.tile([C, 3 * C], FP16, tag="wqs")
        wo16 = P.tile([C, C], FP16, tag="wo16")
        stat = P.tile([C, 4], F32, tag="stat")
        stat16 = P.tile([C, 16], F32, tag="stat16")
        gstat = P.tile([NG, 12], F32, tag="gstat")
        gsti = P.tile([NG, 2], I32, tag="gsti")
        qb = P.tile([C, 1], F32, tag="qb")
        kb = P.tile([C, 1], F32, tag="kb")
        vb = P.tile([C, 1], F32, tag="vb")
        ybias = P.tile([C, 1], F32, tag="ybias")
        warm = P.tile([C, 1], F32, tag="warm")
        d4 = P.tile([C, IC], F32, tag="d4")
        r4 = P.tile([C, IC], FP16, tag="r4")
        re01_sb = P.tile([C, IC], F32, tag="re01_sb")
        re23_sb = P.tile([C, IC], F32, tag="re23_sb")
        osc = P.tile([C, IC], FP16, tag="osc")
        ysb = P.tile([C, IC], F32, tag="ysb")

        if _PFILL or _FILL:
            zfull = P.tile([C, C], FP16, tag="zfull")
            nv.memset(zfull[:], 0.0)

        # pre-warm the exp activation table during the DMAs
        if _WARM == "on":
            nv.memset(warm[:], 0.0)
            ns.activation(warm[:], warm[:], AF.Exp, scale=1.0)

        # ---------------- prologue: GN stats + weight folding + qkv ----
        with tc.tile_pool(name="pps", bufs=2, space="PSUM") as PPS, \
             tc.tile_pool(name="wsc", bufs=2) as WSC:
            if _PFILL:
                # full-array zero passes during the DMA wait: PE cell
                # activity from t~0 so the HAM clock grant arrives early
                pfscr = PPS.tile([C, 512], F32, tag="pfscr")
                for i in range(_PFILL):
                    nt.matmul(pfscr[:, 0:128], zfull[:], zfull[:, 0:C],
                              start=True, stop=True, skip_group_check=True)
            # per-channel sum / sum-of-squares, chunked to overlap the DMA
            for chk in range(8):
                sl = slice(chk * 512, (chk + 1) * 512)
                nv.tensor_reduce(stat16[:, chk:chk + 1], xb[:, sl], AX.X,
                                 ALU.add)
                if _STATS == "ttr":
                    xsq = WSC.tile([C, 512], FP16, tag="xsq")
                    nv.tensor_tensor_reduce(
                        xsq[:], xb[:, sl], xb[:, sl], 1.0, 0.0,
                        ALU.mult, ALU.add, stat16[:, 8 + chk:9 + chk])
                else:
                    xsq = WSC.tile([C, 512], F32, tag="xsqf")
                    nv.tensor_mul(xsq[:], xb[:, sl], xb[:, sl])
                    nv.tensor_reduce(stat16[:, 8 + chk:9 + chk], xsq[:],
                                     AX.X, ALU.add)
            nv.tensor_reduce(stat[:, 0:1], stat16[:, 0:8], AX.X, ALU.add)
            nv.tensor_reduce(stat[:, 1:2], stat16[:, 8:16], AX.X, ALU.add)
            # group-combine via matmul with the group indicator matrix
            gs_p = PPS.tile([NG, 2], F32, tag="tiny")
            nt.matmul(gs_p[:], gmap[:], stat[:, 0:2], start=True, stop=True)
            m_g = gstat[:, 0:1]
            ex2 = gstat[:, 1:2]
            nv.tensor_scalar_mul(m_g, gs_p[:, 0:1], 1.0 / GSIZE)
            nv.tensor_scalar_mul(ex2, gs_p[:, 1:2], 1.0 / GSIZE)
            msq = gstat[:, 2:3]
            nv.tensor_mul(msq, m_g, m_g)
            vare = gstat[:, 3:4]
            nv.tensor_sub(vare, ex2, msq)
            nv.tensor_scalar_add(vare, vare, EPS)   # var + eps
            if _RSQRT == "bit":
                # rsqrt seed via the int-shift trick (no ACT sqrt table)
                nv.tensor_scalar(gsti[:, 0:1], vare.bitcast(I32), 1, None,
                                 ALU.logical_shift_right)
                nv.tensor_scalar(gsti[:, 1:2], gsti[:, 0:1], -1, 0x5f3759df,
                                 ALU.mult, ALU.add)
                r = gsti[:, 1:2].bitcast(F32)
            else:
                sq = gstat[:, 4:5]
                ns.activation(sq, vare, AF.Sqrt)
                r0 = gstat[:, 5:6]
                nv.reciprocal(r0, sq)
                r = r0
            # three Newton steps: r <- r * (1.5 - 0.5 * vare * r^2)
            for it in range(3):
                t1 = gstat[:, 6:7]
                nv.tensor_mul(t1, r, r)
                t2 = gstat[:, 7:8]
                nv.tensor_mul(t2, t1, vare)
                t3 = gstat[:, 8:9]
                nv.tensor_scalar(t3, t2, -0.5, 1.5, ALU.mult, ALU.add)
                rn = gstat[:, 9 + it:10 + it]
                nv.tensor_mul(rn, r, t3)
                r = rn
            # broadcast group mean/rstd back to channels
            st2 = gstat[:, 6:8]
            nv.tensor_copy(st2[:, 0:1], m_g)
            nv.tensor_copy(st2[:, 1:2], r)
            ch_p = PPS.tile([C, 2], F32, tag="tiny")
            nt.matmul(ch_p[:], gmapT[:], st2, start=True, stop=True)
            scale_c = stat[:, 2:3]
            nv.tensor_mul(scale_c, ch_p[:, 1:2], gam)
            tb = stat[:, 3:4]
            nv.tensor_mul(tb, ch_p[:, 0:1], scale_c)
            nv.tensor_sub(tb, bet, tb)

            # fold GN scale into qkv weights (fp16); out-proj weights fp16
            nv.tensor_scalar_mul(wqs[:], wq[:], scale_c)
            nv.tensor_copy(wo16[:], wo[:])
            # qkv biases from the GN shift
            for bi, btile in enumerate((qb, kb, vb)):
                bp = PPS.tile([C, 1], F32, tag="tiny")
                nt.matmul(bp[:], wq[:, bi * C:(bi + 1) * C], tb, start=True,
                          stop=True)
                nv.tensor_copy(btile[:], bp[:])
            ybp = PPS.tile([C, 1], F32, tag="tiny")
            nt.matmul(ybp[:], wo[:], vb[:], start=True, stop=True)
            nv.tensor_add(ybias[:], ybp[:], bout)

            # qT / kT (with folded bias), stacked [(h,d), *], fp16
            for chk in range(IC // 512):
                pq = PPS.tile([C, 512], F32, tag="pq")
                nt.matmul(pq[:], wqs[:, 0:C], xq[:, chk * 512:(chk + 1) * 512],
                          start=True, stop=True)
                nv.tensor_scalar_add(qT[:, chk * 512:(chk + 1) * 512], pq[:],
                                     qb[:])
            for chk in range(S // 512):
                pk = PPS.tile([C, 512], F32, tag="pq")
                nt.matmul(pk[:], wqs[:, C:2 * C], xb[:, chk * 512:(chk + 1) * 512],
                          start=True, stop=True)
                nv.tensor_scalar_add(kT[:, chk * 512:(chk + 1) * 512], pk[:],
                                     kb[:])
            # v in [j, (h,d)] fp16 layout with ones columns interleaved
            # (softmax denominator); 4 j-tiles batched per PSUM tile.
            # PSUM->SBUF copies go on ACT to keep DVE free for the kq casts.
            nv.memset(vaug3[:, :, DH:DH + 1], 1.0)
            for g in range(NJT // 4):
                pv = PPS.tile([C, 512], F32, tag="pq")
                for k in range(4):
                    nt.matmul(pv[:, k * 128:(k + 1) * 128],
                              xb[:, (4 * g + k) * 128:(4 * g + k + 1) * 128],
                              wqs[:, 2 * C:3 * C], start=True, stop=True)
                if _VAUG_ENG == "act":
                    ns.copy(vaug3[:, g * 16:(g + 1) * 16, 0:DH],
                            pv[:].rearrange("p (a d) -> p a d", d=DH))
                else:
                    nv.tensor_copy(vaug3[:, g * 16:(g + 1) * 16, 0:DH],
                                   pv[:].rearrange("p (a d) -> p a d", d=DH))

        # ---------------- attention ----------------
        # Two head-phases: phase 0 runs heads 0,1 over all 32 j-tiles,
        # phase 1 runs heads 2,3. Each phase accumulates AV into a single
        # [128,1024] PSUM tile (2 banks: head-even rows 0:33, head-odd
        # 64:97, denominators from the ones column of v), so the score ring
        # gets SIX banks = 3 tiles deep. With the ring that deep the QK
        # matmuls never wait on the exps - the steady-state period is set
        # by the exp engines (ACT ~1.34us / DVE ~1.47us per [128,1024]
        # tile), not the exp->QK handoff. Phase 0's accumulator evacuates
        # to SBUF at the phase boundary. Full-array zero-matmul fillers
        # keep PE cell activity high so the HAM clock gate grants 2.4 GHz.
        o01sb = P.tile([C, IC], F32, tag="o01sb")
        with tc.tile_pool(name="psc", bufs=3, space="PSUM") as PSC, \
             tc.tile_pool(name="pob", bufs=1, space="PSUM") as POB:

            def emit_fill(ob, n):
                for i in range(n):
                    nt.matmul(ob[:, 0:512], zfull[:], qT[0:C, 0:512],
                              start=False, stop=False, skip_group_check=True)

            def emit_exp(src_ap, eng, name):
                if _EXP_MODE == "act":
                    eng = 'A'
                if eng == 'A':
                    e = EA.tile([C, IC], FP16, tag="Ea", name=f"ea{name}")
                    ns.activation(e[:], src_ap, AF.Exp, scale=SCALE)
                    return ('A', e)
                e = EI.tile([C, IC], I16, tag="Ei", name=f"ei{name}")
                nv.tensor_scalar(e[:], src_ap, SCH_A, SCH_B,
                                 ALU.mult, ALU.add)
                return ('V', e)

            def av_rhs(ent, hf):
                kind, e = ent
                ap = e[:, hf * 512:(hf + 1) * 512]
                return ap.bitcast(FP16) if kind == 'V' else ap

            ob23 = None
            for ph in range(2):
                ob = POB.tile([C, IC], F32, tag="ob", name=f"ob{ph}")
                ha, hb = 2 * ph, 2 * ph + 1

                def emit_av(jt, ea, eb, ob=ob, ha=ha, hb=hb):
                    first, last = jt == 0, jt == NJT - 1
                    for hf in range(2):
                        for base, ent, h in ((0, ea, ha), (64, eb, hb)):
                            nt.matmul(ob[base:base + DH + 1,
                                         hf * 512:(hf + 1) * 512],
                                      vaug3[:, jt * HEADS + h, :],
                                      av_rhs(ent, hf),
                                      start=first, stop=last,
                                      skip_group_check=True,
                                      tile_position=(0, base))

                avq = []
                for jt in range(NJT):
                    sa = PSC.tile([C, IC], F32, tag="sc",
                                  name=f"sa{ph}_{jt}")
                    sb = PSC.tile([C, IC], F32, tag="sc",
                                  name=f"sb{ph}_{jt}")
                    for hf in range(2):
                        for h, sc_ in ((hb, sb), (ha, sa)):
                            nt.matmul(sc_[:, hf * 512:(hf + 1) * 512],
                                      kT[32 * h:32 * (h + 1),
                                         jt * 128:(jt + 1) * 128],
                                      qT[32 * h:32 * (h + 1),
                                         hf * 512:(hf + 1) * 512],
                                      start=True, stop=True,
                                      tile_position=(32 * h, 0))
                    ea = emit_exp(sa[:], 'A', f"{ph}_{jt}a")
                    eb = emit_exp(sb[:], 'V', f"{ph}_{jt}b")
                    avq.append((jt, ea, eb))
                    if _FILL and 1 < jt < NJT - 1:
                        emit_fill(ob, _FILL)
                    if len(avq) > 2:
                        emit_av(*avq.pop(0))
                while avq:
                    emit_av(*avq.pop(0))
                if ph == 0:
                    # evacuate phase-0 accumulator so its banks free up
                    # (only the written partition ranges: head rows + denom)
                    ns.copy(o01sb[0:DH + 1, :], ob[0:DH + 1, :])
                    nv.tensor_copy(o01sb[64:64 + DH + 1, :],
                                   ob[64:64 + DH + 1, :])
                else:
                    ob23 = ob

            # ---------------- epilogue ----------------
            # denominators land at ob partitions DH / 64+DH; gather them to
            # partitions 0/32/64/96 of d4 (rest stays 1.0 for the reciprocal)
            nv.memset(d4[:], 1.0)
            ns.copy(d4[0:1, :], o01sb[DH:DH + 1, :])
            nv.tensor_copy(d4[32:33, :], o01sb[64 + DH:64 + DH + 1, :])
            ns.copy(d4[64:65, :], ob23[DH:DH + 1, :])
            nv.tensor_copy(d4[96:97, :], ob23[64 + DH:64 + DH + 1, :])
            r4f = P.tile([C, IC], F32, tag="r4f")
            nv.reciprocal(r4f[:], d4[:])
            nv.tensor_copy(r4[:], r4f[:])
            # broadcast reciprocals into the accumulators' own partition
            # layout (heads at rows 0:32 / 64:96 of each half) so the DVE
            # multiplies see equal base partitions on both inputs
            re01t = PSC.tile([C, IC], F32, tag="sc")
            re23t = PSC.tile([C, IC], F32, tag="sc")
            re01 = re01t[:]
            re23 = re23t[:]
            for hf in range(2):
                nt.matmul(re01t[:, hf * 512:(hf + 1) * 512], bmap[:, 0:C],
                          r4[:, hf * 512:(hf + 1) * 512], start=True, stop=True)
                nt.matmul(re23t[:, hf * 512:(hf + 1) * 512], bmap[:, C:2 * C],
                          r4[:, hf * 512:(hf + 1) * 512], start=True, stop=True)
            if _EPI == "v2":
                ns.copy(re01_sb[:], re01)
            else:
                nv.tensor_copy(re01_sb[:], re01)
            nv.tensor_copy(re23_sb[:], re23)
            nv.tensor_mul(osc[0:32, :], o01sb[0:32, :], re01_sb[0:32, :])
            nv.tensor_mul(osc[32:64, :], o01sb[64:96, :], re01_sb[64:96, :])
            nv.tensor_mul(osc[64:96, :], ob23[0:32, :], re23_sb[0:32, :])
            nv.tensor_mul(osc[96:128, :], ob23[64:96, :], re23_sb[64:96, :])
            ypsum = POB.tile([C, IC], F32, tag="ob")
            for hf in range(2):
                nt.matmul(ypsum[:, hf * 512:(hf + 1) * 512], wo16[:],
                          osc[:, hf * 512:(hf + 1) * 512], start=True,
                          stop=True)
            nv.tensor_scalar_add(ysb[:], ypsum[:], ybias[:])
            nc.sync.dma_start(d_y[:], ysb[:])

    nc.compile()   # bacc passes: split sync waits (HW: 1 wait/inst), DCE
    return nc


_PROG = None


def _get_prog():
    global _PROG
    if _PROG is None:
        _PROG = _build()
    return _PROG


def _in_maps(x, gn_gamma, gn_beta, w_qkv, w_out, b_out):
    x = np.asarray(x, dtype=np.float32)
    gmap = np.zeros((C, NG), dtype=np.float32)
    gmap[np.arange(C), np.arange(C) // (C // NG)] = 1.0
    # cols 0:128 build re01 (h0 -> rows 0:32, h1 -> rows 64:96),
    # cols 128:256 build re23 (h2 -> rows 0:32, h3 -> rows 64:96);
    # reciprocal sources sit at partitions 0/32/64/96 of r4
    bmap = np.zeros((C, 2 * C), dtype=np.float16)
    bmap[0, 0:32] = 1.0
    bmap[32, 64:96] = 1.0
    bmap[64, C + 0:C + 32] = 1.0
    bmap[96, C + 64:C + 96] = 1.0
    base = dict(
        wqkvT=np.ascontiguousarray(np.asarray(w_qkv, np.float32).T),
        woutT=np.ascontiguousarray(np.asarray(w_out, np.float32).T),
        gam=np.asarray(gn_gamma, np.float32).reshape(C, 1),
        bet=np.asarray(gn_beta, np.float32).reshape(C, 1),
        bout=np.asarray(b_out, np.float32).reshape(C, 1),
        gmap=gmap,
        gmapT=np.ascontiguousarray(gmap.T),
        bmap=bmap,
    )
    maps = []
    for core in range(8):
        b, qc = core // 4, core % 4
        xb = np.ascontiguousarray(x[b].reshape(C, S).astype(np.float16))
        m = dict(base)
        m["xb"] = xb
        m["xq"] = np.ascontiguousarray(xb[:, qc * IC:(qc + 1) * IC])
        maps.append(m)
    return maps


def kernel(x, gn_gamma, gn_beta, w_qkv, w_out, b_out):
    nc = _get_prog()
    maps = _in_maps(x, gn_gamma, gn_beta, w_qkv, w_out, b_out)
    res = run_bass_kernel_spmd(nc, maps, list(range(8))).results
    y = np.empty((2, C, S), dtype=np.float32)
    for core in range(8):
        b, qc = core // 4, core % 4
        y[b, :, qc * IC:(qc + 1) * IC] = res[core]["y"]
    return y.reshape(2, C, 16, 16, 16)


# revision 31
# speedup vs baseline: 1.3616x; 1.1295x over previous
"""AttnBlock (GroupNorm + 4-head self-attention + out-proj) on 8 trn2 cores.

Sharding: core = (batch b in 0..1) x (query-quarter qc in 0..3).
Each core computes the full pipeline for its batch and its 1024-query slice.

v2 design (vs the 426us fp32-QK baseline):
  - ALL matmuls run fp16 (fp32 streams 4 cycles/col on the PE; fp16 streams 1).
    x ships from host as fp16; GN folds into fp16 qkv weights; kT/qT/v are
    fp16; scores accumulate fp32 in PSUM.
  - QK^T row-tiles head pairs at tile_position (32h, 0) so the two K=32
    matmuls of a pair ride the same PE pass (fp16 weights occupy one row
    group, unlike fp32 hi/lo which spilled into the partner's rows).
  - exp is the hard floor (16.8M elems at 1 elem/cycle/lane on ACT): the
    score tiles are split between the Scalar engine (exact exp) and the
    Vector engine (Schraudolph 2^t: int16(t*1024 + 15316) bit-cast to fp16,
    one tensor_scalar from PSUM; +-3% sawtooth cancels in softmax renorm to
    ~7e-3 final rel err at a 44% share).
  - AV accumulates all 32 j-tiles directly in PSUM (start at jt0, stop at
    jt31) - no block merges on DVE. The ones column appended to v yields the
    softmax denominator for free.
  - rsqrt for GN uses the int-shift seed + Newton on DVE, so the ACT sqrt
    table never loads and the exp table (pre-warmed at t=0) stays resident.
Output slices are disjoint, so the host just concatenates 8 results.
"""

import numpy as np
from contextlib import ExitStack

import concourse.bass as bass
import concourse.mybir as mybir
import concourse.tile as tile
from concourse import bacc
from concourse.bass_utils import run_bass_kernel_spmd

F32 = mybir.dt.float32
FP16 = mybir.dt.float16
I16 = mybir.dt.int16
I32 = mybir.dt.int32
AF = mybir.ActivationFunctionType
ALU = mybir.AluOpType
AX = mybir.AxisListType

HEADS, DH = 4, 32
C = 128           # channels == HEADS*DH
S = 4096          # spatial f*h*w
IC = 1024         # queries per core
NJT = S // 128    # 32 key tiles
SCALE = DH ** -0.5
EPS = 1e-5
NG = 32           # groupnorm groups
GSIZE = (C // NG) * S  # elements per group

LOG2E = 1.4426950408889634
SCH_A = SCALE * LOG2E * 1024.0   # Schraudolph multiplier (raw score -> i16)
SCH_B = 15360.0 - 44.0           # exponent bias - relative-error centering

import os
_EXP_MODE = os.environ.get("K_EXP", "mixed")      # mixed | act
_VAUG_ENG = os.environ.get("K_VAUG", "act")       # act | dve
_RSQRT = os.environ.get("K_RSQRT", "bit")         # bit | act
_AV_ACCUM = os.environ.get("K_AV", "full")        # full | block
# tensor_tensor_reduce passes CoreSim but crashes the HW exec unit
_STATS = os.environ.get("K_STATS", "base")        # base | ttr (ttr: HW crash)
_EPI = os.environ.get("K_EPI", "v2")              # v2 | base
_WARM = os.environ.get("K_WARM", "on")            # on | off
_SC16 = os.environ.get("K_SC16", "off")           # on: fp16 score tiles
_FILL = int(os.environ.get("K_FILL", "1"))        # PE-warming filler MMs/jt
_PFILL = int(os.environ.get("K_PFILL", "0"))      # prologue filler MM count


def _exp_eng(p, half):
    """Engine for exp tile `half` of pair p: 'A' (scalar/ACT) or 'V' (DVE).
    Fixed per score buffer: sa (half 0) always ACT, sb (half 1) always DVE.
    The QK matmul writing sb is emitted first each pair: its WAR dependency
    (the slower DVE read of the previous pair) resolves last, so the sa
    matmul queues right behind it and rides the same PE pass."""
    if _EXP_MODE == "act":
        return 'A'
    return 'A' if half == 0 else 'V'


def _build():
    nc = bacc.Bacc("TRN2", target_bir_lowering=False)
    d_xb = nc.declare_dram_parameter("xb", [C, S], FP16, isOutput=False)
    d_xq = nc.declare_dram_parameter("xq", [C, IC], FP16, isOutput=False)
    d_wqkv = nc.declare_dram_parameter("wqkvT", [C, 3 * C], F32, isOutput=False)
    d_wout = nc.declare_dram_parameter("woutT", [C, C], F32, isOutput=False)
    d_gam = nc.declare_dram_parameter("gam", [C, 1], F32, isOutput=False)
    d_bet = nc.declare_dram_parameter("bet", [C, 1], F32, isOutput=False)
    d_bout = nc.declare_dram_parameter("bout", [C, 1], F32, isOutput=False)
    d_gmap = nc.declare_dram_parameter("gmap", [C, NG], F32, isOutput=False)
    d_gmapT = nc.declare_dram_parameter("gmapT", [NG, C], F32, isOutput=False)
    d_bmap = nc.declare_dram_parameter("bmap", [C, 2 * C], FP16, isOutput=False)
    d_y = nc.declare_dram_parameter("y", [C, IC], F32, isOutput=True)

    with tile.TileContext(nc) as tc, ExitStack() as ctx:
        nv, ns, nt = nc.vector, nc.scalar, nc.tensor
        P = ctx.enter_context(tc.tile_pool(name="persist", bufs=1))
        EA = ctx.enter_context(tc.tile_pool(name="ea", bufs=6))
        EI = ctx.enter_context(tc.tile_pool(name="ei", bufs=6))

        # ---------------- loads (chunked so stats overlap the DMA) ------
        xb = P.tile([C, S], FP16, tag="xb")
        for chk in range(8):
            sl = slice(chk * 512, (chk + 1) * 512)
            nc.sync.dma_start(xb[:, sl], d_xb[:, sl])
        xq = P.tile([C, IC], FP16, tag="xq")
        nc.sync.dma_start(xq[:], d_xq[:])
        wq = P.tile([C, 3 * C], F32, tag="wq")
        nc.sync.dma_start(wq[:], d_wqkv[:])
        wo = P.tile([C, C], F32, tag="wo")
        nc.sync.dma_start(wo[:], d_wout[:])
        gbb = P.tile([C, 3], F32, tag="gbb")
        nc.sync.dma_start(gbb[:, 0:1], d_gam[:])
        nc.sync.dma_start(gbb[:, 1:2], d_bet[:])
        nc.sync.dma_start(gbb[:, 2:3], d_bout[:])
        gam, bet, bout = gbb[:, 0:1], gbb[:, 1:2], gbb[:, 2:3]
        gmap = P.tile([C, NG], F32, tag="gmap")
        nc.sync.dma_start(gmap[:], d_gmap[:])
        gmapT = P.tile([NG, C], F32, tag="gmapT")
        nc.sync.dma_start(gmapT[:], d_gmapT[:])
        bmap = P.tile([C, 2 * C], FP16, tag="bmap")
        nc.sync.dma_start(bmap[:], d_bmap[:])

        # persistent products
        kT = P.tile([C, S], FP16, tag="kT")        # [(h,d), j]
        qT = P.tile([C, IC], FP16, tag="qT")       # [(h,d), i]
        vaug = P.tile([C, NJT * HEADS * (DH + 1)], FP16, tag="vaug")
        vaug3 = vaug[:].rearrange("p (a b) -> p a b", b=DH + 1)  # a = jt*4+h
        wqs = P.tile([C, 3 * C], FP16, tag="wqs")
        wo16 = P.tile([C, C], FP16, tag="wo16")
        stat = P.tile([C, 4], F32, tag="stat")
        stat16 = P.tile([C, 16], F32, tag="stat16")
        gstat = P.tile([NG, 12], F32, tag="gstat")
        gsti = P.tile([NG, 2], I32, tag="gsti")
        qb = P.tile([C, 1], F32, tag="qb")
        kb = P.tile([C, 1], F32, tag="kb")
        vb = P.tile([C, 1], F32, tag="vb")
        ybias = P.tile([C, 1], F32, tag="ybias")
        warm = P.tile([C, 1], F32, tag="warm")
        d4 = P.tile([C, IC], F32, tag="d4")
        r4 = P.tile([C, IC], FP16, tag="r4")
        re01_sb = P.tile([C, IC], F32, tag="re01_sb")
        re23_sb = P.tile([C, IC], F32, tag="re23_sb")
        osc = P.tile([C, IC], FP16, tag="osc")
        ysb = P.tile([C, IC], F32, tag="ysb")

        if _PFILL or _FILL:
            zfull = P.tile([C, C], FP16, tag="zfull")
            nv.memset(zfull[:], 0.0)

        # pre-warm the exp activation table during the DMAs
        if _WARM == "on":
            nv.memset(warm[:], 0.0)
            ns.activation(warm[:], warm[:], AF.Exp, scale=1.0)

        # ---------------- prologue: GN stats + weight folding + qkv ----
        with tc.tile_pool(name="pps", bufs=2, space="PSUM") as PPS, \
             tc.tile_pool(name="wsc", bufs=2) as WSC:
            if _PFILL:
                # full-array zero passes during the DMA wait: PE cell
                # activity from t~0 so the HAM clock grant arrives early
                pfscr = PPS.tile([C, 512], F32, tag="pfscr")
                for i in range(_PFILL):
                    nt.matmul(pfscr[:, 0:128], zfull[:], zfull[:, 0:C],
                              start=True, stop=True, skip_group_check=True)
            # per-channel sum / sum-of-squares, chunked to overlap the DMA
            for chk in range(8):
                sl = slice(chk * 512, (chk + 1) * 512)
                nv.tensor_reduce(stat16[:, chk:chk + 1], xb[:, sl], AX.X,
                                 ALU.add)
                if _STATS == "ttr":
                    xsq = WSC.tile([C, 512], FP16, tag="xsq")
                    nv.tensor_tensor_reduce(
                        xsq[:], xb[:, sl], xb[:, sl], 1.0, 0.0,
                        ALU.mult, ALU.add, stat16[:, 8 + chk:9 + chk])
                else:
                    xsq = WSC.tile([C, 512], F32, tag="xsqf")
                    nv.tensor_mul(xsq[:], xb[:, sl], xb[:, sl])
                    nv.tensor_reduce(stat16[:, 8 + chk:9 + chk], xsq[:],
                                     AX.X, ALU.add)
            nv.tensor_reduce(stat[:, 0:1], stat16[:, 0:8], AX.X, ALU.add)
            nv.tensor_reduce(stat[:, 1:2], stat16[:, 8:16], AX.X, ALU.add)
            # group-combine via matmul with the group indicator matrix
            gs_p = PPS.tile([NG, 2], F32, tag="tiny")
            nt.matmul(gs_p[:], gmap[:], stat[:, 0:2], start=True, stop=True)
            m_g = gstat[:, 0:1]
            ex2 = gstat[:, 1:2]
            nv.tensor_scalar_mul(m_g, gs_p[:, 0:1], 1.0 / GSIZE)
            nv.tensor_scalar_mul(ex2, gs_p[:, 1:2], 1.0 / GSIZE)
            msq = gstat[:, 2:3]
            nv.tensor_mul(msq, m_g, m_g)
            vare = gstat[:, 3:4]
            nv.tensor_sub(vare, ex2, msq)
            nv.tensor_scalar_add(vare, vare, EPS)   # var + eps
            if _RSQRT == "bit":
                # rsqrt seed via the int-shift trick (no ACT sqrt table)
                nv.tensor_scalar(gsti[:, 0:1], vare.bitcast(I32), 1, None,
                                 ALU.logical_shift_right)
                nv.tensor_scalar(gsti[:, 1:2], gsti[:, 0:1], -1, 0x5f3759df,
                                 ALU.mult, ALU.add)
                r = gsti[:, 1:2].bitcast(F32)
            else:
                sq = gstat[:, 4:5]
                ns.activation(sq, vare, AF.Sqrt)
                r0 = gstat[:, 5:6]
                nv.reciprocal(r0, sq)
                r = r0
            # three Newton steps: r <- r * (1.5 - 0.5 * vare * r^2)
            for it in range(3):
                t1 = gstat[:, 6:7]
                nv.tensor_mul(t1, r, r)
                t2 = gstat[:, 7:8]
                nv.tensor_mul(t2, t1, vare)
                t3 = gstat[:, 8:9]
                nv.tensor_scalar(t3, t2, -0.5, 1.5, ALU.mult, ALU.add)
                rn = gstat[:, 9 + it:10 + it]
                nv.tensor_mul(rn, r, t3)
                r = rn
            # broadcast group mean/rstd back to channels
            st2 = gstat[:, 6:8]
            nv.tensor_copy(st2[:, 0:1], m_g)
            nv.tensor_copy(st2[:, 1:2], r)
            ch_p = PPS.tile([C, 2], F32, tag="tiny")
            nt.matmul(ch_p[:], gmapT[:], st2, start=True, stop=True)
            scale_c = stat[:, 2:3]
            nv.tensor_mul(scale_c, ch_p[:, 1:2], gam)
            tb = stat[:, 3:4]
            nv.tensor_mul(tb, ch_p[:, 0:1], scale_c)
            nv.tensor_sub(tb, bet, tb)

            # fold GN scale into qkv weights (fp16); out-proj weights fp16
            nv.tensor_scalar_mul(wqs[:], wq[:], scale_c)
            nv.tensor_copy(wo16[:], wo[:])
            # qkv biases from the GN shift
            for bi, btile in enumerate((qb, kb, vb)):
                bp = PPS.tile([C, 1], F32, tag="tiny")
                nt.matmul(bp[:], wq[:, bi * C:(bi + 1) * C], tb, start=True,
                          stop=True)
                nv.tensor_copy(btile[:], bp[:])
            ybp = PPS.tile([C, 1], F32, tag="tiny")
            nt.matmul(ybp[:], wo[:], vb[:], start=True, stop=True)
            nv.tensor_add(ybias[:], ybp[:], bout)

            # qT / kT (with folded bias), stacked [(h,d), *], fp16
            for chk in range(IC // 512):
                pq = PPS.tile([C, 512], F32, tag="pq")
                nt.matmul(pq[:], wqs[:, 0:C], xq[:, chk * 512:(chk + 1) * 512],
                          start=True, stop=True)
                nv.tensor_scalar_add(qT[:, chk * 512:(chk + 1) * 512], pq[:],
                                     qb[:])
            for chk in range(S // 512):
                pk = PPS.tile([C, 512], F32, tag="pq")
                nt.matmul(pk[:], wqs[:, C:2 * C], xb[:, chk * 512:(chk + 1) * 512],
                          start=True, stop=True)
                nv.tensor_scalar_add(kT[:, chk * 512:(chk + 1) * 512], pk[:],
                                     kb[:])
            # v in [j, (h,d)] fp16 layout with ones columns interleaved
            # (softmax denominator); 4 j-tiles batched per PSUM tile.
            # PSUM->SBUF copies go on ACT to keep DVE free for the kq casts.
            nv.memset(vaug3[:, :, DH:DH + 1], 1.0)
            for g in range(NJT // 4):
                pv = PPS.tile([C, 512], F32, tag="pq")
                for k in range(4):
                    nt.matmul(pv[:, k * 128:(k + 1) * 128],
                              xb[:, (4 * g + k) * 128:(4 * g + k + 1) * 128],
                              wqs[:, 2 * C:3 * C], start=True, stop=True)
                if _VAUG_ENG == "act":
                    ns.copy(vaug3[:, g * 16:(g + 1) * 16, 0:DH],
                            pv[:].rearrange("p (a d) -> p a d", d=DH))
                else:
                    nv.tensor_copy(vaug3[:, g * 16:(g + 1) * 16, 0:DH],
                                   pv[:].rearrange("p (a d) -> p a d", d=DH))

        # ---------------- attention ----------------
        # Two head-phases: phase 0 runs heads 0,1 over all 32 j-tiles,
        # phase 1 runs heads 2,3. Each phase accumulates AV into a single
        # [128,1024] PSUM tile (2 banks: head-even rows 0:33, head-odd
        # 64:97, denominators from the ones column of v), so the score ring
        # gets SIX banks = 3 tiles deep. With the ring that deep the QK
        # matmuls never wait on the exps - the steady-state period is set
        # by the exp engines (ACT ~1.34us / DVE ~1.47us per [128,1024]
        # tile), not the exp->QK handoff. Phase 0's accumulator evacuates
        # to SBUF at the phase boundary. Full-array zero-matmul fillers
        # keep PE cell activity high so the HAM clock gate grants 2.4 GHz.
        o01sb = P.tile([C, IC], F32, tag="o01sb")
        with tc.tile_pool(name="psc", bufs=3, space="PSUM") as PSC, \
             tc.tile_pool(name="pob", bufs=1, space="PSUM") as POB:

            def emit_fill(ob, n):
                for i in range(n):
                    nt.matmul(ob[:, 0:512], zfull[:], qT[0:C, 0:512],
                              start=False, stop=False, skip_group_check=True)

            def emit_exp(src_ap, eng, name):
                if _EXP_MODE == "act":
                    eng = 'A'
                if eng == 'A':
                    e = EA.tile([C, IC], FP16, tag="Ea", name=f"ea{name}")
                    ns.activation(e[:], src_ap, AF.Exp, scale=SCALE)
                    return ('A', e)
                e = EI.tile([C, IC], I16, tag="Ei", name=f"ei{name}")
                nv.tensor_scalar(e[:], src_ap, SCH_A, SCH_B,
                                 ALU.mult, ALU.add)
                return ('V', e)

            def av_rhs(ent, hf):
                kind, e = ent
                ap = e[:, hf * 512:(hf + 1) * 512]
                return ap.bitcast(FP16) if kind == 'V' else ap

            ob23 = None
            for ph in range(2):
                ob = POB.tile([C, IC], F32, tag="ob", name=f"ob{ph}")
                ha, hb = 2 * ph, 2 * ph + 1

                def emit_av(jt, ea, eb, ob=ob, ha=ha, hb=hb):
                    first, last = jt == 0, jt == NJT - 1
                    for hf in range(2):
                        for base, ent, h in ((0, ea, ha), (64, eb, hb)):
                            nt.matmul(ob[base:base + DH + 1,
                                         hf * 512:(hf + 1) * 512],
                                      vaug3[:, jt * HEADS + h, :],
                                      av_rhs(ent, hf),
                                      start=first, stop=last,
                                      skip_group_check=True,
                                      tile_position=(0, base))

                avq = []
                for jt in range(NJT):
                    sa = PSC.tile([C, IC], F32, tag="sc",
                                  name=f"sa{ph}_{jt}")
                    sb = PSC.tile([C, IC], F32, tag="sc",
                                  name=f"sb{ph}_{jt}")
                    for hf in range(2):
                        for h, sc_ in ((hb, sb), (ha, sa)):
                            nt.matmul(sc_[:, hf * 512:(hf + 1) * 512],
                                      kT[32 * h:32 * (h + 1),
                                         jt * 128:(jt + 1) * 128],
                                      qT[32 * h:32 * (h + 1),
                                         hf * 512:(hf + 1) * 512],
                                      start=True, stop=True,
                                      tile_position=(32 * h, 0))
                    ea = emit_exp(sa[:], 'A', f"{ph}_{jt}a")
                    eb = emit_exp(sb[:], 'V', f"{ph}_{jt}b")
                    avq.append((jt, ea, eb))
                    if _FILL and 1 < jt < NJT - 1:
                        emit_fill(ob, _FILL)
                    if len(avq) > 2:
                        emit_av(*avq.pop(0))
                while avq:
                    emit_av(*avq.pop(0))
                if ph == 0:
                    # evacuate phase-0 accumulator so its banks free up
                    # (only the written partition ranges: head rows + denom)
                    ns.copy(o01sb[0:DH + 1, :], ob[0:DH + 1, :])
                    nv.tensor_copy(o01sb[64:64 + DH + 1, :],
                                   ob[64:64 + DH + 1, :])
                else:
                    ob23 = ob

            # ---------------- epilogue ----------------
            # denominators land at ob partitions DH / 64+DH; gather them to
            # partitions 0/32/64/96 of d4 (rest stays 1.0 for the reciprocal)
            nv.memset(d4[:], 1.0)
            ns.copy(d4[0:1, :], o01sb[DH:DH + 1, :])
            nv.tensor_copy(d4[32:33, :], o01sb[64 + DH:64 + DH + 1, :])
            ns.copy(d4[64:65, :], ob23[DH:DH + 1, :])
            nv.tensor_copy(d4[96:97, :], ob23[64 + DH:64 + DH + 1, :])
            r4f = P.tile([C, IC], F32, tag="r4f")
            nv.reciprocal(r4f[:], d4[:])
            nv.tensor_copy(r4[:], r4f[:])
            # broadcast reciprocals into the accumulators' own partition
            # layout (heads at rows 0:32 / 64:96 of each half) so the DVE
            # multiplies see equal base partitions on both inputs
            re01t = PSC.tile([C, IC], F32, tag="sc")
            re23t = PSC.tile([C, IC], F32, tag="sc")
            re01 = re01t[:]
            re23 = re23t[:]
            for hf in range(2):
                nt.matmul(re01t[:, hf * 512:(hf + 1) * 512], bmap[:, 0:C],
                          r4[:, hf * 512:(hf + 1) * 512], start=True, stop=True)
                nt.matmul(re23t[:, hf * 512:(hf + 1) * 512], bmap[:, C:2 * C],
                          r4[:, hf * 512:(hf + 1) * 512], start=True, stop=True)
            if _EPI == "v2":
                ns.copy(re01_sb[:], re01)
            else:
                nv.tensor_copy(re01_sb[:], re01)
            nv.tensor_copy(re23_sb[:], re23)
            nv.tensor_mul(osc[0:32, :], o01sb[0:32, :], re01_sb[0:32, :])
            nv.tensor_mul(osc[32:64, :], o01sb[64:96, :], re01_sb[64:96, :])
            nv.tensor_mul(osc[64:96, :], ob23[0:32, :], re23_sb[0:32, :])
            nv.tensor_mul(osc[96:128, :], ob23[64:96, :], re23_sb[64:96, :])
            ypsum = POB.tile([C, IC], F32, tag="ob")
            for hf in range(2):
                nt.matmul(ypsum[:, hf * 512:(hf + 1) * 512], wo16[:],
                          osc[:, hf * 512:(hf + 1) * 512], start=True,
                          stop=True)
            nv.tensor_scalar_add(ysb[:], ypsum[:], ybias[:])
            nc.sync.dma_start(d_y[:], ysb[:])

    nc.compile()   # bacc passes: split sync waits (HW: 1 wait/inst), DCE
    return nc


_PROG = None


def _get_prog():
    global _PROG
    if _PROG is None:
        _PROG = _build()
    return _PROG


def _in_maps(x, gn_gamma, gn_beta, w_qkv, w_out, b_out):
    x = np.asarray(x, dtype=np.float32)
    gmap = np.zeros((C, NG), dtype=np.float32)
    gmap[np.arange(C), np.arange(C) // (C // NG)] = 1.0
    # cols 0:128 build re01 (h0 -> rows 0:32, h1 -> rows 64:96),
    # cols 128:256 build re23 (h2 -> rows 0:32, h3 -> rows 64:96);
    # reciprocal sources sit at partitions 0/32/64/96 of r4
    bmap = np.zeros((C, 2 * C), dtype=np.float16)
    bmap[0, 0:32] = 1.0
    bmap[32, 64:96] = 1.0
    bmap[64, C + 0:C + 32] = 1.0
    bmap[96, C + 64:C + 96] = 1.0
    base = dict(
        wqkvT=np.ascontiguousarray(np.asarray(w_qkv, np.float32).T),
        woutT=np.ascontiguousarray(np.asarray(w_out, np.float32).T),
        gam=np.asarray(gn_gamma, np.float32).reshape(C, 1),
        bet=np.asarray(gn_beta, np.float32).reshape(C, 1),
        bout=np.asarray(b_out, np.float32).reshape(C, 1),
        gmap=gmap,
        gmapT=np.ascontiguousarray(gmap.T),
        bmap=bmap,
    )
    maps = []
    for core in range(8):
        b, qc = core // 4, core % 4
        xb = np.ascontiguousarray(x[b].reshape(C, S).astype(np.float16))
        m = dict(base)
        m["xb"] = xb
        m["xq"] = np.ascontiguousarray(xb[:, qc * IC:(qc + 1) * IC])
        maps.append(m)
    return maps


def kernel(x, gn_gamma, gn_beta, w_qkv, w_out, b_out):
    nc = _get_prog()
    maps = _in_maps(x, gn_gamma, gn_beta, w_qkv, w_out, b_out)
    res = run_bass_kernel_spmd(nc, maps, list(range(8))).results
    y = np.empty((2, C, S), dtype=np.float32)
    for core in range(8):
        b, qc = core // 4, core % 4
        y[b, :, qc * IC:(qc + 1) * IC] = res[core]["y"]
    return y.reshape(2, C, 16, 16, 16)
